# revision 55
# baseline (speedup 1.0000x reference)
"""Trainium2 Bass kernel for a hybrid classical/quantum head.

Math: the reference is  out = Q(tanh(X @ Wpre.T + bpre) * pi/2) @ Wpost.T + bpost
where Q() simulates a 10-qubit circuit: H on all wires, per-sample RY(theta_w),
then 6 layers of (CNOT chain + shared RY(qw)), returning PauliZ expvals.

Restructuring used here:
  * After H + per-sample RY, the state is a PRODUCT state:
      s2[j] = prod_w v_w(bit_w(j)),  v_w(0)=cos(phi_w), v_w(1)=sin(phi_w),
      phi_w = theta_w/2 + pi/4,  theta_w = tanh(pre)*pi/2.
    All v are strictly positive (phi in (0, pi/2)), so the product state can
    be built in AMPLITUDE-MAJOR layout directly via a log-domain matmul:
      S2^T[k, s] = exp( sum_r Bits[k, r] * ln(v_r[s]) )
    with Bits a constant (1024, 20) 0/1 bit-selection matrix.  This kills
    the 64 serialized DMA transposes that dominated the previous version.
  * Everything after the per-sample RY layer is a fixed linear operator A
    (1024x1024) depending only on q_params -> built host-side in fp64,
    shipped as fp16.
  * z_w = sum_j sign_w(j) * (A s2)_j^2, and the post-linear folds in:
      out[s, c] = sum_j d[c, j] * y[s, j]^2 + bpost[c],  d = Wpost @ Sgn.

Per-core device pipeline (data-parallel over batch, 1024 samples/core):
  prenet matmul (Wpre stationary, fp16 PE) -> [20,1024] tanh/sin/ln ACT chain
  -> Bits matmul (PE) -> exp (ACT) -> S2^T fp16
  -> Y^T = A @ S2^T (fp16 PE matmul, f32 accum) -> square (ACT)
  -> d-contraction (fp16 PE matmul) -> +bias (DVE) -> out^T f32.

Scheduling notes (each worth microseconds on the 65us critical path):
  * only 2 ACT table loads (tanh/sin in silu set, ln/exp/square/identity in
    natural_log_exp set -- forced via _patch_act_tables), first one hoisted
    off-path by a dummy tanh.
  * DMA dispatches serialize on the Sync DGE and completions count into ONE
    cumulative semaphore, so dma_starts are issued in dependency-topology
    order (blobs, x ch0, x ch1, A).
  * PE warm-up matmuls bridge the DMA wait and the ACT head chain so the
    p-state never drops before the main matmul grind.
  * bits matmul runs 2 kt ahead of 4 interleaved jt chains, hiding the
    ~0.9us EXP latency; chunk 1's bits/exp pairs hide inside chunk 0's
    remaining chains.
  * x arrives in non-uniform quarters (384/256/256/128) feeding the prenet
    progressively; Ln runs in halves with its bias AP data-forced behind the
    last sin (via a DVE min()) so chunk 0's bits matmul starts 0.4us sooner
    without the scheduler breaking the two ACT table phases.
"""

import numpy as np

N_QUBITS = 10
Q_DEPTH = 6
MAX_LAYERS = 15
DIM = 2**N_QUBITS
N_CORES = 8
B_FULL = 8192
F_IN = 512
N_CLS = 2
BC = B_FULL // N_CORES  # 1024 samples per core
P = 128

_CACHE = {}


def _build_A(q_params):
    """Fixed circuit operator after the per-sample RY layer, fp64 on host."""
    qp = np.asarray(q_params, np.float64)
    qw = qp.reshape(MAX_LAYERS, N_QUBITS)
    N = N_QUBITS

    def apply_1q(M, U, w):
        a, b = 2**w, 2 ** (N - 1 - w)
        M = M.reshape(a, 2, b, DIM)
        M = np.einsum('ij,ajbk->aibk', U, M)
        return M.reshape(DIM, DIM)

    def apply_cnot(M, c, t):
        M = M.reshape(2**c, 2, 2 ** (t - c - 1), 2, 2 ** (N - 1 - t), DIM)
        M = np.stack([M[:, 0], np.flip(M[:, 1], axis=2)], axis=1)
        return M.reshape(DIM, DIM)

    def ry(th):
        c, s = np.cos(th / 2), np.sin(th / 2)
        return np.array([[c, -s], [s, c]])

    A = np.eye(DIM)
    for k in range(Q_DEPTH):
        for i in range(0, N - 1, 2):
            A = apply_cnot(A, i, i + 1)
        for i in range(1, N - 1, 2):
            A = apply_cnot(A, i, i + 1)
        for w in range(N):
            A = apply_1q(A, ry(qw[k + 1, w]), w)
    return A


NKT = DIM // P  # 8 amplitude tiles
NFT = F_IN // P  # 4 feature tiles
NCH = 2  # two 512-wide sample chunks (PSUM bank = 512 f32)
CW = BC // NCH  # 512
NW = 2 * N_QUBITS  # 20 selection rows
# x/prenet pipeline quarters, non-uniform: the transfer is serial so the
# LAST quarter gates the tanh/sin chain -- make it smallest, front-load the
# first quarter into the NEFF-preamble overlap window.
QXW = [384, 256, 256, 128]
QXO = [0, 384, 640, 896]          # sample-column offsets
NQX = len(QXW)
XSEG = [NFT * o for o in QXO]     # flat x segment offsets (per partition)


def _patch_act_tables():
    """Force the act-table-load pass to use only the two co-resident sets we
    need: silu_and_others (tanh+sin) and natural_log_exp_and_others
    (ln+exp+square+identity).  The default per-activation greedy choice loads
    4 tables (~1.3us each, serialized on the ACT engine's critical path).
    Entry order (= act_func_set_id) is preserved; unwanted sets are emptied."""
    import concourse.bacc as bacc_mod
    from concourse.hw_specs import get_activation_tables as _orig

    if getattr(bacc_mod.get_activation_tables, "_act_filtered", False):
        return
    keep = {"silu_and_others", "natural_log_exp_and_others"}

    def _filtered(arch):
        return {
            name: (s if name in keep else set())
            for name, s in _orig(arch).items()
        }

    _filtered._act_filtered = True
    bacc_mod.get_activation_tables = _filtered


def _build_bass():
    import concourse.mybir as mybir
    from concourse import bacc
    from concourse.tile import TileContext

    _patch_act_tables()
    dt = mybir.dt
    AF = mybir.ActivationFunctionType
    PI = float(np.pi)

    nc = bacc.Bacc()
    # blob16 packs (per-partition, fp16): wpre [*, 4*20], bits [*, 8*128]
    # (junk below row 20), d [*, 8*2].  blob32 packs the [20, 1] f32 bias
    # columns: bpre, bsin, eps, bpost (rows 0-1).  One dma_start each --
    # DIRECT2D dispatches serialize at ~650ns on the Sync sequencer, so
    # fewer, larger DMAs shorten the head.
    W_OFF, B_OFF, D_OFF = 0, NFT * NW, NFT * NW + DIM
    BLOB16_W = D_OFF + NKT * N_CLS
    blob16 = nc.dram_tensor("blob16", [P, BLOB16_W], dt.float16, kind="ExternalInput")
    blob32 = nc.dram_tensor("blob32", [NW, 4], dt.float32, kind="ExternalInput")
    xin = nc.dram_tensor("xin", [P, NFT * BC], dt.float16, kind="ExternalInput")
    a16 = nc.dram_tensor("a16", [P, NKT, DIM], dt.float16, kind="ExternalInput")
    outT = nc.dram_tensor("outT", [N_CLS, BC], dt.float32, kind="ExternalOutput")

    with TileContext(nc) as tc:
        with (
            tc.tile_pool(name="const", bufs=1) as cpool,
            tc.tile_pool(name="ps_a", bufs=4, space="PSUM") as ps_a,
            tc.tile_pool(name="ps_y", bufs=4, space="PSUM") as ps_y,
        ):
            # ---- inputs to SBUF, all on the Sync DGE (dispatching from the
            # ACT DGE steals ACT-engine cycles and round-robins against the
            # Sync queue's descriptors).  x first, split by sample chunk so
            # prenet(ch0) runs while x(ch1) is in flight; tiny blobs next;
            # A (not needed until the main matmul) last, FIFO behind x ----
            # The DMA-completion semaphore is a single cumulative counter:
            # waiting on the Nth dma_start transitively waits on all N-1
            # before it.  So dispatch strictly in dependency-topology order:
            # blobs (wpre/biases, tiny), x ch0, x ch1, then A.
            x_sb = cpool.tile([P, NFT * BC], dt.float16)

            def xseg(qx):
                return slice(XSEG[qx], XSEG[qx] + NFT * QXW[qx])

            nc.sync.dma_start(x_sb[:, xseg(0)], xin[:, xseg(0)])
            blob16_sb = cpool.tile([P, BLOB16_W], dt.float16)
            nc.sync.dma_start(blob16_sb, blob16[:])
            blob32_sb = cpool.tile([NW, 4], dt.float32)
            nc.sync.dma_start(blob32_sb, blob32[:])
            for qx in range(1, NQX):
                nc.sync.dma_start(x_sb[:, xseg(qx)], xin[:, xseg(qx)])
            a_sb = cpool.tile([P, NKT, DIM], dt.float16)
            nc.sync.dma_start(a_sb, a16[:])

            def wpre_ap(ft):
                return blob16_sb[:, W_OFF + ft * NW:W_OFF + (ft + 1) * NW]

            def bits_ap(kt):
                return blob16_sb[0:NW, B_OFF + kt * P:B_OFF + (kt + 1) * P]

            def d_ap(jt):
                return blob16_sb[:, D_OFF + jt * N_CLS:D_OFF + (jt + 1) * N_CLS]

            bpre_b = blob32_sb[:, 0:1]
            bsin_b = blob32_sb[:, 1:2]
            eps_b = blob32_sb[:, 2:3]
            bpost_b = blob32_sb[0:N_CLS, 3:4]

            # ---- PE warm-up: dummy matmuls ramp the p-state and keep the
            # PE busy until x arrives (idle resets the ramp); a dummy tanh
            # pulls the silu-set ACT_TABLE_LOAD off the critical path (it
            # only depends on the local memset) ----
            warm_in = cpool.tile([P, 2 * P], dt.float16)
            nc.gpsimd.memset(warm_in, 0.0)
            warm_ps = ps_y.tile([P, 2 * P], dt.float32, name="warm", tag="y")
            for _ in range(14):
                nc.tensor.matmul(warm_ps, warm_in[:, 0:P], warm_in)
            dummy_act = cpool.tile([1, 1], dt.float32)
            nc.scalar.activation(dummy_act, warm_in[0:1, 0:1], AF.Tanh)

            q20 = cpool.tile([NW, BC], dt.float32)
            v20 = cpool.tile([NW, BC], dt.float32)
            lv20 = cpool.tile([NW, BC], dt.float16)
            s2T = cpool.tile([P, NKT, BC], dt.float16)
            p16 = cpool.tile([P, NKT, BC], dt.float16)
            outT_sb = cpool.tile([N_CLS, BC], dt.float32)

            # ---- prenet: pre[q, s] = Wpre @ X^T (q duplicated 2x), +bpre,
            # then tanh/sin per chunk (all silu-set ops together), then the
            # single table switch, then ln per chunk (ln(ch1) is only
            # needed much later, but keeping it here keeps ACT order simple;
            # it runs while the PE does the ch0 bits matmuls) ----
            for qx in range(NQX):
                w = QXW[qx]
                qsl = slice(QXO[qx], QXO[qx] + w)
                pre_ps = ps_a.tile([NW, w], dt.float32, name=f"pre{qx}", tag="ps")
                for ft in range(NFT):
                    fo = XSEG[qx] + ft * w
                    nc.tensor.matmul(
                        pre_ps, wpre_ap(ft), x_sb[:, fo:fo + w],
                        start=(ft == 0), stop=(ft == NFT - 1),
                    )
                nc.scalar.activation(
                    q20[:, qsl], pre_ps, AF.Tanh, bias=bpre_b
                )
                # rows 0-9: cos(phi) = sin(pi/4 q + 3pi/4); 10-19: sin(phi)
                nc.scalar.activation(
                    v20[:, qsl], q20[:, qsl], AF.Sin, bias=bsin_b, scale=PI / 4.0
                )
            # keep the PE p-state up through the ACT head chain
            for _ in range(8):
                nc.tensor.matmul(warm_ps, warm_in[:, 0:P], warm_in)
            # lv = ln(v + 1e-5), fp16, split in halves so the ch0 bits
            # matmul starts after only 512 columns.  The bias AP is routed
            # through a DVE min() that reads sin-q3's last output column
            # (min(1e-5, v) == 1e-5 always since v > 1e-4): this DATA-forces
            # both Ln halves after the whole silu phase, so the scheduler
            # cannot interleave them with tanh/sin (which would cost two
    	    # extra ACT table loads).
            eps2 = cpool.tile([NW, 1], dt.float32)
            nc.vector.tensor_scalar(
                eps2, v20[:, BC - 1:BC], eps_b, None, mybir.AluOpType.min
            )
            nc.scalar.activation(lv20[:, 0:CW], v20[:, 0:CW], AF.Ln, bias=eps2[:, 0:1])
            nc.scalar.activation(lv20[:, CW:BC], v20[:, CW:BC], AF.Ln, bias=eps2[:, 0:1])

            # ---- S2^T = exp(Bits @ lv) fused with the main matmul.
            # The bits matmul for (ch, kt) must precede its EXP, and with
            # only 2 ps_a banks, issuing all 16 up-front makes the PE crawl
            # at the ACT engine's EXP cadence.  Instead:
            #   * ch0: kt-outer fusion -- bits-mm(kt), exp(kt), then the
            #     kt-step of 4 interleaved jt chains (4 matmuls per EXP,
            #     so the PE never waits after pipeline fill).
            #   * ch0 jt4-7: normal chains, with ch1's bits-mm/exp pairs
            #     slotted between chains (PE work hides the pool WAR).
            #   * ch1: normal chains at full speed.
            csl0 = slice(0, CW)
            csl1 = slice(CW, BC)
            NIL = 4  # interleaved chains

            def bits_exp(ch, kt):
                csl = csl0 if ch == 0 else csl1
                s2log = ps_a.tile([P, CW], dt.float32, name=f"s2l{ch}_{kt}", tag="ps")
                nc.tensor.matmul(s2log, bits_ap(kt), lv20[:, csl])
                nc.scalar.activation(s2T[:, kt, csl], s2log, AF.Exp)

            y_il = [
                ps_y.tile([P, CW], dt.float32, name=f"y0_{jt}", tag="y")
                for jt in range(NIL)
            ]
            # bits-mm runs one kt AHEAD of the il chains, so each EXP's
            # ~0.9us latency is hidden behind the previous kt's 4 matmuls
            bits_exp(0, 0)
            bits_exp(0, 1)
            for kt in range(NKT):
                if kt + 2 < NKT:
                    bits_exp(0, kt + 2)
                for jt in range(NIL):
                    nc.tensor.matmul(
                        y_il[jt], a_sb[:, kt, jt * P:(jt + 1) * P],
                        s2T[:, kt, csl0],
                        start=(kt == 0), stop=(kt == NKT - 1),
                        skip_group_check=True,
                    )
            for jt in range(NIL):
                nc.scalar.activation(p16[:, jt, csl0], y_il[jt], AF.Square)

            for jt in range(NIL, NKT):
                jsl = slice(jt * P, (jt + 1) * P)
                y_ps = ps_y.tile([P, CW], dt.float32, name=f"y0_{jt}", tag="y")
                for kt in range(NKT):
                    nc.tensor.matmul(
                        y_ps, a_sb[:, kt, jsl], s2T[:, kt, csl0],
                        start=(kt == 0), stop=(kt == NKT - 1),
                    )
                nc.scalar.activation(p16[:, jt, csl0], y_ps, AF.Square)
                kb = 2 * (jt - NIL)
                bits_exp(1, kb)
                bits_exp(1, kb + 1)

            for jt in range(NKT):
                jsl = slice(jt * P, (jt + 1) * P)
                y_ps = ps_y.tile([P, CW], dt.float32, name=f"y1_{jt}", tag="y")
                for kt in range(NKT):
                    nc.tensor.matmul(
                        y_ps, a_sb[:, kt, jsl], s2T[:, kt, csl1],
                        start=(kt == 0), stop=(kt == NKT - 1),
                    )
                nc.scalar.activation(p16[:, jt, csl1], y_ps, AF.Square)

            # ---- out^T = d @ (Y^T)^2, bias on the (otherwise idle) DVE;
            # ch0's result DMAs out while the PE still works on ch1 ----
            for ch in range(NCH):
                csl = slice(ch * CW, (ch + 1) * CW)
                out_ps = ps_a.tile([N_CLS, CW], dt.float32, name=f"o{ch}", tag="ps")
                for jt in range(NKT):
                    nc.tensor.matmul(
                        out_ps, d_ap(jt), p16[:, jt, csl],
                        start=(jt == 0), stop=(jt == NKT - 1),
                    )
                nc.vector.tensor_scalar_add(outT_sb[:, csl], out_ps, bpost_b)
                nc.sync.dma_start(outT[:, csl], outT_sb[:, csl])

    nc.finalize()
    return nc


def _get_nc():
    if "nc" not in _CACHE:
        _CACHE["nc"] = _build_bass()
    return _CACHE["nc"]


def _pack_pmajor(arr, p=P):
    """(R, C) -> (p, R//p, C) partition-major: row r -> [r % ... ] tiles of p."""
    R, C = arr.shape
    return np.ascontiguousarray(
        arr.reshape(R // p, p, C).transpose(1, 0, 2)
    )


def _prepare_in_maps(input_features, W_pre, b_pre, q_params, W_post, b_post):
    X = np.asarray(input_features, np.float32)
    A = _build_A(q_params)
    aT16 = _pack_pmajor(np.ascontiguousarray(A.T).astype(np.float16))  # [128,8,1024]

    j = np.arange(DIM)
    bitmat = np.stack(
        [(j >> (N_QUBITS - 1 - w)) & 1 for w in range(N_QUBITS)]
    )  # (10, 1024)
    bits = np.concatenate([1.0 - bitmat, bitmat], axis=0).astype(np.float16)  # (20,1024)
    sgn = 1.0 - 2.0 * bitmat
    d = np.asarray(W_post, np.float64) @ sgn  # (2, 1024)
    dT16 = _pack_pmajor(np.ascontiguousarray(d.T).astype(np.float16))  # [128,8,2]

    WT = np.asarray(W_pre, np.float16).T  # (512, 10)
    wpre_pack = _pack_pmajor(
        np.ascontiguousarray(np.concatenate([WT, WT], axis=1))
    )  # [128,4,20]

    # blob16: [128, 4*20 | 8*128 (bits, junk rows >= 20) | 8*2 (d)]
    bits_pad = np.zeros((P, DIM), np.float16)
    bits_pad[0:NW, :] = bits
    blob16 = np.concatenate([
        wpre_pack.reshape(P, NFT * NW),
        bits_pad,
        dT16.reshape(P, NKT * N_CLS),
    ], axis=1)
    blob16 = np.ascontiguousarray(blob16)

    # blob32 columns: bpre (2x), bsin, eps, bpost (rows 0-1)
    blob32 = np.zeros((NW, 4), np.float32)
    blob32[:, 0] = np.tile(np.asarray(b_pre, np.float32), 2)
    blob32[0:N_QUBITS, 1] = 3 * np.pi / 4
    blob32[N_QUBITS:NW, 1] = np.pi / 4
    blob32[:, 2] = 1e-5
    blob32[0:N_CLS, 3] = np.asarray(b_post, np.float32)

    XT16 = np.asarray(X, np.float16).T  # (512, 8192)
    in_maps = []
    for c in range(N_CORES):
        xc = _pack_pmajor(
            np.ascontiguousarray(XT16[:, c * BC:(c + 1) * BC])
        )  # [128,4,1024]
        xin = np.concatenate(
            [
                xc[:, :, QXO[q]:QXO[q] + QXW[q]].reshape(P, NFT * QXW[q])
                for q in range(NQX)
            ],
            axis=1,
        )  # [128, 4096] flat segments
        xin = np.ascontiguousarray(xin)
        in_maps.append({
            "xin": xin,
            "blob16": blob16,
            "blob32": blob32,
            "a16": aT16,
        })
    return in_maps


def run(inputs, trace=False):
    """Run on 8 cores; returns (output (8192, 2) f32, BassKernelResults)."""
    from concourse.bass_utils import run_bass_kernel_spmd

    nc = _get_nc()
    in_maps = _prepare_in_maps(**inputs)
    res = run_bass_kernel_spmd(
        nc, in_maps, core_ids=list(range(N_CORES)), trace=trace
    )
    out = np.empty((B_FULL, N_CLS), np.float32)
    for c in range(N_CORES):
        out[c * BC:(c + 1) * BC, :] = res.results[c]["outT"].T
    return out, res


def kernel(input_features, W_pre, b_pre, q_params, W_post, b_post):
    out, _ = run(dict(
        input_features=input_features, W_pre=W_pre, b_pre=b_pre,
        q_params=q_params, W_post=W_post, b_post=b_post,
    ))
    return out
